# revision 11
# baseline (speedup 1.0000x reference)
"""DNGPU cell (gated conv recurrence) for Trainium2, data-parallel over batch on 8 cores.

Problem: B=32, L=128, C=192, K=3; 128 sequential steps of
    reset = sigmoid(conv(mem, w_reset) + 0.5)
    gate  = sigmoid(conv(mem, w_gate) + 0.7)
    cand  = tanh(conv(reset*mem, w_cand))
    mem   = gate*shift_right(mem) + (1-gate)*cand

Per-core layout: state held in SBUF as [C partitions, token cols] where
token col = 4 + l*4 + b (l-major, b-minor, 4 zero-pad cols each side), so
conv taps are +-4-column-shifted views. C=192 splits into chA (0:128) and
chB (128:192); chB is stored duplicated: memB rows 0:64 = chB, rows
64:128 = chB shifted left 4 cols, so all conv contractions use full
128-row moving operands:
  M0/M1/M2 = memA[:, k*4 : k*4+512]        (taps 0..2, chA)
  M3       = memB[:, 0:512]                (rows 0:64 tap0, 64:128 tap1)
  M4       = memB[:, 4:516]                (rows 64:128 tap2; weight rows
                                            0:64 are zero)
Five 128x128 stationary chunks x {T0=resetA, T1=gateA, T2=[resetB|gateB],
C0=candA, C1=candB(zero-padded cols)} = 25 uniform fp32r matmuls per step
(full PE rate at N>=256). reset/gate sigmoids use fused per-partition
bias tiles. Elementwise work is split across DVE (nc.vector) and Pool
(nc.gpsimd) so neither gates the PE.
"""

import numpy as np
from contextlib import ExitStack

import concourse.bacc as bacc
import concourse.tile as tile
from concourse import mybir
from concourse.bass_utils import run_bass_kernel_spmd

B, L, C = 32, 128, 192
NCORES = 8
BLOC = B // NCORES          # 4 batches per core
TOK = BLOC * L              # 512 tokens per core
WPAD = TOK + 8              # 4 zero cols each side
STEPS = 128

F32 = mybir.dt.float32
F32R = mybir.dt.float32r
AF = mybir.ActivationFunctionType
SUB = mybir.AluOpType.subtract
MULT = mybir.AluOpType.mult


def build(steps=STEPS):
    nc = bacc.Bacc("TRN2", target_bir_lowering=False, debug=False,
                   num_devices=NCORES)
    x_d = nc.dram_tensor("x", [BLOC, L, C], F32, kind="ExternalInput").ap()
    w_d = {}
    b_d = {}
    for cv, wn, bn in (("r", "w_reset", "b_reset"),
                       ("g", "w_gate", "b_gate"),
                       ("n", "w_cand", "b_cand")):
        w_d[cv] = nc.dram_tensor(wn, [3, C, C], F32, kind="ExternalInput").ap()
        b_d[cv] = nc.dram_tensor(bn, [C], F32, kind="ExternalInput").ap()
    id_d = nc.dram_tensor("ident", [128, 128], F32, kind="ExternalInput").ap()
    out_d = nc.dram_tensor("out", [BLOC, L, C], F32, kind="ExternalOutput").ap()

    with tile.TileContext(nc) as tc, ExitStack() as ctx:
        const = ctx.enter_context(tc.tile_pool(name="const", bufs=1))
        state = ctx.enter_context(tc.tile_pool(name="state", bufs=1))
        act = ctx.enter_context(tc.tile_pool(name="act", bufs=2))
        tmp = ctx.enter_context(tc.tile_pool(name="tmp", bufs=2))
        psum = ctx.enter_context(tc.tile_pool(name="psum", bufs=1, space="PSUM"))

        # --- stationary weight tiles ------------------------------------
        # rg tiles: T0 -> reset chA outs, T1 -> gate chA outs,
        #           T2 -> [reset chB | gate chB] outs.
        # cand tiles: C0 -> cand chA, C1 -> [cand chB | zeros].
        # chunk rows: c0/c1/c2 = tap c x chA; c3 = [tap0 chB; tap1 chB];
        #             c4 = [zeros; tap2 chB].
        zsrc = const.tile([64, 128], F32, tag="zsrc", name="zsrc")
        nc.gpsimd.memset(zsrc[:], 0.0)

        def load_chunk_rows(t, cv, outslice, col0, ncol):
            o0, o1 = outslice
            for c in range(3):
                nc.gpsimd.dma_start(t[c][:, col0:col0 + ncol],
                                    w_d[cv][c, 0:128, o0:o1])
            nc.gpsimd.dma_start(t[3][0:64, col0:col0 + ncol],
                                w_d[cv][0, 128:192, o0:o1])
            nc.gpsimd.dma_start(t[3][64:128, col0:col0 + ncol],
                                w_d[cv][1, 128:192, o0:o1])
            nc.gpsimd.dma_start(t[4][64:128, col0:col0 + ncol],
                                w_d[cv][2, 128:192, o0:o1])

        wt = {}
        for name in ("T0", "T1", "T2", "C0", "C1"):
            wt[name] = [const.tile([128, 128], F32R, tag=f"w{name}{c}",
                                   name=f"w{name}{c}") for c in range(5)]
            # zero rows 0:64 of chunk 4 (and padding cols of C1)
            nc.vector.tensor_copy(wt[name][4][0:64, :], zsrc[:])
            if name == "C1":
                for c in range(5):
                    nc.vector.tensor_copy(wt[name][c][0:64, 64:128],
                                          zsrc[0:64, 0:64])
                    nc.vector.tensor_copy(wt[name][c][64:128, 64:128],
                                          zsrc[0:64, 0:64])
        load_chunk_rows(wt["T0"], "r", (0, 128), 0, 128)
        load_chunk_rows(wt["T1"], "g", (0, 128), 0, 128)
        load_chunk_rows(wt["T2"], "r", (128, 192), 0, 64)
        load_chunk_rows(wt["T2"], "g", (128, 192), 64, 64)
        load_chunk_rows(wt["C0"], "n", (0, 128), 0, 128)
        load_chunk_rows(wt["C1"], "n", (128, 192), 0, 64)

        # --- bias tiles --------------------------------------------------
        bA = const.tile([128, 1], F32, tag="bA")
        nc.sync.dma_start(bA[:, 0], b_d["r"][0:128])
        bG = const.tile([128, 1], F32, tag="bG")
        nc.sync.dma_start(bG[:, 0], b_d["g"][0:128])
        bB = const.tile([128, 1], F32, tag="bB")
        nc.sync.dma_start(bB[0:64, 0], b_d["r"][128:192])
        nc.sync.dma_start(bB[64:128, 0], b_d["g"][128:192])
        bCA = const.tile([128, 1], F32, tag="bCA")
        nc.sync.dma_start(bCA[:, 0], b_d["n"][0:128])
        bCB = const.tile([128, 1], F32, tag="bCB")
        nc.sync.dma_start(bCB[0:64, 0], b_d["n"][128:192])
        nc.vector.tensor_copy(bCB[64:128, 0:1], zsrc[0:64, 0:1])
        # negated gateB bias for omgB = sigmoid(-z - b) = 1 - gateB
        bGnB = const.tile([128, 1], F32, tag="bGnB")
        nc.sync.dma_start(bGnB[64:128, 0], b_d["g"][128:192])
        nc.vector.tensor_scalar_mul(bGnB[64:128, 0:1], bGnB[64:128, 0:1], -1.0)

        ident = const.tile([128, 128], F32, tag="ident")
        nc.sync.dma_start(ident[:], id_d)
        identr = const.tile([128, 128], F32R, tag="identr")
        nc.gpsimd.dma_start(identr[:], id_d)

        # --- state tiles -------------------------------------------------
        memA = {i: state.tile([128, WPAD], F32R, tag=f"memA{i}", name=f"memA{i}")
                for i in range(2)}
        memB = {i: state.tile([128, WPAD], F32R, tag=f"memB{i}", name=f"memB{i}")
                for i in range(2)}
        rmemA = state.tile([128, WPAD], F32R, tag="rmemA", name="rmemA")
        rmemB = state.tile([128, WPAD], F32R, tag="rmemB", name="rmemB")
        zf32 = state.tile([128, WPAD], F32, tag="zf32", name="zf32")
        nc.gpsimd.memset(zf32[:], 0.0)
        for t in (memA[0], memA[1], memB[0], memB[1], rmemA, rmemB):
            nc.vector.tensor_copy(t[:], zf32[:])

        # --- input transform: x[b,l,c] -> mem[0] = [c, 4+l*4+b] ----------
        for b in range(BLOC):
            xb = tmp.tile([L, C], F32, tag="xload")
            nc.sync.dma_start(xb[:], x_d[b])
            ps = psum.tile([128, L], F32, tag="tpF32")
            nc.tensor.transpose(ps[:], xb[:, 0:128], ident[:])
            nc.vector.tensor_copy(memA[0][:, 4 + b: 4 + b + 4 * L: 4], ps[:])
            ps2 = psum.tile([128, L], F32, tag="tpF32")
            nc.tensor.transpose(ps2[0:64, :], xb[:, 128:192], ident[:])
            nc.vector.tensor_copy(memB[0][0:64, 4 + b: 4 + b + 4 * L: 4],
                                  ps2[0:64, :])
        # shifted duplicate for chB
        nc.vector.tensor_copy(memB[0][64:128, 0:TOK], memB[0][0:64, 4:4 + TOK])

        # --- recurrence --------------------------------------------------
        MWIN = ((0, 0), (1, 4), (2, 8))  # (chunk idx, col offset) chA taps

        cur = 0
        for t in range(steps):
            mA, mB = memA[cur], memB[cur]
            nA, nB = memA[1 - cur], memB[1 - cur]

            pT0 = psum.tile([128, TOK], F32, tag="pT0", name="pT0")
            pT1 = psum.tile([128, TOK], F32, tag="pT1", name="pT1")
            pT2 = psum.tile([128, TOK], F32, tag="pT2", name="pT2")
            pC0 = psum.tile([128, TOK], F32, tag="pC0", name="pC0")
            pC1 = psum.tile([128, TOK], F32, tag="pC1", name="pC1")

            def rg_cha(p, wts):
                for c, off in MWIN:
                    nc.tensor.matmul(p[:], wts[c][:], mA[:, off:off + TOK],
                                     start=(c == 0), stop=False)

            def rg_chb(p, wts):
                nc.tensor.matmul(p[:], wts[3][:], mB[:, 0:TOK],
                                 start=False, stop=False)
                nc.tensor.matmul(p[:], wts[4][:], mB[:, 4:4 + TOK],
                                 start=False, stop=True)

            # rg matmuls: chA chunks first (depend only on subA of prev
            # step), chB chunks later (depend on subB+dup chain).
            rg_cha(pT0, wt["T0"])
            rg_cha(pT2, wt["T2"])
            rg_chb(pT0, wt["T0"])
            rg_chb(pT2, wt["T2"])
            rg_cha(pT1, wt["T1"])
            rg_chb(pT1, wt["T1"])

            # activations (ACT queue order = emission order)
            sA = act.tile([128, TOK], F32R, tag="sA", name="sA")
            nc.scalar.activation(sA[:], pT0[:], AF.Sigmoid, bias=bA[:, 0:1])
            sBr = act.tile([64, TOK], F32R, tag="sBr", name="sBr")
            nc.scalar.activation(sBr[:], pT2[0:64, :], AF.Sigmoid,
                                 bias=bB[0:64, 0:1])
            # gateB / (1-gateB) aligned to partitions 0:64
            sGb = act.tile([64, TOK], F32R, tag="sGb", name="sGb")
            nc.scalar.activation(sGb[:], pT2[64:128, :], AF.Sigmoid,
                                 bias=bB[64:128, 0:1])
            oGb = act.tile([64, TOK], F32R, tag="oGb", name="oGb")
            nc.scalar.activation(oGb[:], pT2[64:128, :], AF.Sigmoid,
                                 bias=bGnB[64:128, 0:1], scale=-1.0)
            sG = act.tile([128, TOK], F32R, tag="sG", name="sG")
            nc.scalar.activation(sG[:], pT1[:], AF.Sigmoid, bias=bG[:, 0:1])

            # rmem = sigmoid(reset) * mem   (Vector: chA + dup, Pool: chB)
            nc.vector.tensor_mul(rmemA[:, 4:4 + TOK], sA[:], mA[:, 4:4 + TOK])
            nc.gpsimd.tensor_mul(rmemB[0:64, 4:4 + TOK], sBr[:],
                                 mB[0:64, 4:4 + TOK])
            # shifted dup: rmemB rows 64:128 = rmemB rows 0:64 shifted left 4
            nc.vector.tensor_copy(rmemB[64:128, 0:TOK], rmemB[0:64, 4:4 + TOK])

            # u = gate * shift_right(mem)
            uA = tmp.tile([128, TOK], F32R, tag="uA", name="uA")
            nc.vector.tensor_mul(uA[:], sG[:], mA[:, 0:TOK])
            uB = tmp.tile([64, TOK], F32R, tag="uB", name="uB")
            nc.gpsimd.tensor_mul(uB[:], sGb[:], mB[0:64, 0:TOK])

            # cand conv
            for c, off in MWIN:
                nc.tensor.matmul(pC0[:], wt["C0"][c][:],
                                 rmemA[:, off:off + TOK],
                                 start=(c == 0), stop=False)
            nc.tensor.matmul(pC0[:], wt["C0"][3][:], rmemB[:, 0:TOK],
                             start=False, stop=False)
            nc.tensor.matmul(pC0[:], wt["C0"][4][:], rmemB[:, 4:4 + TOK],
                             start=False, stop=True)
            for c, off in MWIN:
                nc.tensor.matmul(pC1[:], wt["C1"][c][:],
                                 rmemA[:, off:off + TOK],
                                 start=(c == 0), stop=False)
            nc.tensor.matmul(pC1[:], wt["C1"][3][:], rmemB[:, 0:TOK],
                             start=False, stop=False)
            nc.tensor.matmul(pC1[:], wt["C1"][4][:], rmemB[:, 4:4 + TOK],
                             start=False, stop=True)

            cA = act.tile([128, TOK], F32R, tag="cA", name="cA")
            nc.scalar.activation(cA[:], pC0[:], AF.Tanh, bias=bCA[:, 0:1])
            cB = act.tile([64, TOK], F32R, tag="cB", name="cB")
            nc.scalar.activation(cB[:], pC1[0:64, :], AF.Tanh, bias=bCB[0:64, 0:1])

            # mem_next = u + (1-gate)*cand
            qA = tmp.tile([128, TOK], F32R, tag="qA", name="qA")
            nc.vector.scalar_tensor_tensor(qA[:], sG[:], 1.0, cA[:],
                                           op0=SUB, op1=MULT)
            nc.vector.tensor_sub(nA[:, 4:4 + TOK], uA[:], qA[:])
            qB = tmp.tile([64, TOK], F32R, tag="qB", name="qB")
            nc.gpsimd.tensor_mul(qB[:], oGb[:], cB[:])
            nc.gpsimd.tensor_add(nB[0:64, 4:4 + TOK], uB[:], qB[:])
            nc.vector.tensor_copy(nB[64:128, 0:TOK], nB[0:64, 4:4 + TOK])

            cur = 1 - cur

        # --- output transform: mem[cur] -> out[b,l,c] --------------------
        for b in range(BLOC):
            osb = tmp.tile([L, C], F32, tag="oload")
            ps = psum.tile([L, 128], F32R, tag="tpR")
            nc.tensor.transpose(ps[:], memA[cur][:, 4 + b: 4 + b + 4 * L: 4],
                                identr[:])
            nc.vector.tensor_copy(osb[:, 0:128], ps[:])
            ps2 = psum.tile([L, 128], F32R, tag="tpR")
            nc.tensor.transpose(ps2[:, 0:64], memB[cur][0:64, 4 + b: 4 + b + 4 * L: 4],
                                identr[0:64, 0:64])
            nc.vector.tensor_copy(osb[:, 128:192], ps2[:, 0:64])
            nc.sync.dma_start(out_d[b], osb[:])

    nc.compile()
    return nc


_built = {}


def _get(steps=STEPS):
    if steps not in _built:
        _built[steps] = build(steps)
    return _built[steps]


def kernel(x, w_reset, b_reset, w_gate, b_gate, w_cand, b_cand, steps=STEPS,
           trace=False):
    nc = _get(steps)
    ident = np.eye(128, dtype=np.float32)
    base = {"w_reset": np.asarray(w_reset, np.float32),
            "b_reset": np.asarray(b_reset, np.float32),
            "w_gate": np.asarray(w_gate, np.float32),
            "b_gate": np.asarray(b_gate, np.float32),
            "w_cand": np.asarray(w_cand, np.float32),
            "b_cand": np.asarray(b_cand, np.float32),
            "ident": ident}
    x = np.asarray(x, np.float32)
    in_maps = [dict(base, x=np.ascontiguousarray(x[i * BLOC:(i + 1) * BLOC]))
               for i in range(NCORES)]
    res = run_bass_kernel_spmd(nc, in_maps, core_ids=list(range(NCORES)),
                               trace=trace)
    out = np.concatenate([res.results[i]["out"] for i in range(NCORES)], axis=0)
    if trace:
        return out, res
    return out


if __name__ == "__main__":
    rng = np.random.default_rng(0)
    scale = 1.0 / np.sqrt(3 * C)
    ins = {
        "x": rng.standard_normal((B, L, C), dtype=np.float32),
        "w_reset": (rng.standard_normal((3, C, C)) * scale).astype(np.float32),
        "b_reset": np.full(C, 0.5, np.float32),
        "w_gate": (rng.standard_normal((3, C, C)) * scale).astype(np.float32),
        "b_gate": np.full(C, 0.7, np.float32),
        "w_cand": (rng.standard_normal((3, C, C)) * scale).astype(np.float32),
        "b_cand": np.zeros(C, np.float32),
    }
    out = kernel(**ins, steps=2)
    print("smoke ok", out.shape, out.dtype)


# revision 16
# speedup vs baseline: 1.2852x; 1.2852x over previous
"""DNGPU cell for Trainium2 — 8 cores data-parallel over batch, and within
each core the 4 local batches split into TWO independent pipelines of 2
batches each. The two pipelines are staggered on the PE (P0.rg, P1.rg,
P0.cand, P1.cand per step) so each pipeline's sigmoid/rmem/tanh/combine
chains run while the PE works on the other pipeline — the PE never waits
on a serial chain.

Per-pipeline layout: [C partitions, cols], col = 2 + 2*l + j (l-major,
j = local batch 0/1), 2 zero-pad cols left, 4 right (WPADP=262). Conv
taps are +-2-column shifts. chB (ch 128:192) is stored duplicated:
memB rows 0:64 = chB, rows 64:128 = chB shifted left 2 (dup col c =
main col c+2), so all contractions are full 128-row moving operands:
  M0/M1/M2 = memA[:, 2k : 2k+256]   (taps, chA)
  M3       = memB[:, 0:256]         (rows 0:64 tap0, 64:128 tap1)
  M4       = memB[:, 2:258]         (rows 64:128 tap2; rows 0:64 zero w)
25 uniform 128x128xN=256 fp32r matmuls per pipeline per step (fp32r is
full rate at N>=256). Elementwise: chain ops on DVE, slack-rich ops
(rmem muls, u muls) on gpsimd.
"""

import numpy as np
from contextlib import ExitStack

import concourse.bacc as bacc
import concourse.tile as tile
from concourse import mybir
from concourse.bass_utils import run_bass_kernel_spmd

B, L, C = 32, 128, 192
NCORES = 8
BLOC = B // NCORES          # 4 batches per core
NP = 2                      # pipelines per core
TOKP = 2 * L                # 256 tokens per pipeline
WPADP = TOKP + 6            # 2 zero cols left, 4 right
STEPS = 128

F32 = mybir.dt.float32
F32R = mybir.dt.float32r
AF = mybir.ActivationFunctionType
SUB = mybir.AluOpType.subtract
MULT = mybir.AluOpType.mult


def build(steps=STEPS):
    nc = bacc.Bacc("TRN2", target_bir_lowering=False, debug=False,
                   num_devices=NCORES)
    x_d = nc.dram_tensor("x", [BLOC, L, C], F32, kind="ExternalInput").ap()
    w_d = {}
    b_d = {}
    for cv, wn, bn in (("r", "w_reset", "b_reset"),
                       ("g", "w_gate", "b_gate"),
                       ("n", "w_cand", "b_cand")):
        w_d[cv] = nc.dram_tensor(wn, [3, C, C], F32, kind="ExternalInput").ap()
        b_d[cv] = nc.dram_tensor(bn, [C], F32, kind="ExternalInput").ap()
    id_d = nc.dram_tensor("ident", [128, 128], F32, kind="ExternalInput").ap()
    out_d = nc.dram_tensor("out", [BLOC, L, C], F32, kind="ExternalOutput").ap()

    with tile.TileContext(nc) as tc, ExitStack() as ctx:
        const = ctx.enter_context(tc.tile_pool(name="const", bufs=1))
        state = ctx.enter_context(tc.tile_pool(name="state", bufs=1))
        act = ctx.enter_context(tc.tile_pool(name="act", bufs=2))
        tmp = ctx.enter_context(tc.tile_pool(name="tmp", bufs=2))
        psum = ctx.enter_context(tc.tile_pool(name="psum", bufs=1, space="PSUM"))

        # --- stationary weight tiles (shared by both pipelines) ----------
        zsrc = const.tile([64, 128], F32, tag="zsrc", name="zsrc")
        nc.gpsimd.memset(zsrc[:], 0.0)

        def load_chunk_rows(t, cv, outslice, col0, ncol):
            o0, o1 = outslice
            for c in range(3):
                nc.gpsimd.dma_start(t[c][:, col0:col0 + ncol],
                                    w_d[cv][c, 0:128, o0:o1])
            nc.gpsimd.dma_start(t[3][0:64, col0:col0 + ncol],
                                w_d[cv][0, 128:192, o0:o1])
            nc.gpsimd.dma_start(t[3][64:128, col0:col0 + ncol],
                                w_d[cv][1, 128:192, o0:o1])
            nc.gpsimd.dma_start(t[4][64:128, col0:col0 + ncol],
                                w_d[cv][2, 128:192, o0:o1])

        wt = {}
        for name in ("T0", "T1", "T2", "C0", "C1"):
            wt[name] = [const.tile([128, 128], F32R, tag=f"w{name}{c}",
                                   name=f"w{name}{c}") for c in range(5)]
            nc.vector.tensor_copy(wt[name][4][0:64, :], zsrc[:])
            if name == "C1":
                for c in range(5):
                    nc.vector.tensor_copy(wt[name][c][0:64, 64:128],
                                          zsrc[0:64, 0:64])
                    nc.vector.tensor_copy(wt[name][c][64:128, 64:128],
                                          zsrc[0:64, 0:64])
        load_chunk_rows(wt["T0"], "r", (0, 128), 0, 128)
        load_chunk_rows(wt["T1"], "g", (0, 128), 0, 128)
        load_chunk_rows(wt["T2"], "r", (128, 192), 0, 64)
        load_chunk_rows(wt["T2"], "g", (128, 192), 64, 64)
        load_chunk_rows(wt["C0"], "n", (0, 128), 0, 128)
        load_chunk_rows(wt["C1"], "n", (128, 192), 0, 64)

        # --- bias tiles --------------------------------------------------
        bA = const.tile([128, 1], F32, tag="bA")
        nc.sync.dma_start(bA[:, 0], b_d["r"][0:128])
        bG = const.tile([128, 1], F32, tag="bG")
        nc.sync.dma_start(bG[:, 0], b_d["g"][0:128])
        bB = const.tile([128, 1], F32, tag="bB")
        nc.sync.dma_start(bB[0:64, 0], b_d["r"][128:192])
        nc.sync.dma_start(bB[64:128, 0], b_d["g"][128:192])
        bCA = const.tile([128, 1], F32, tag="bCA")
        nc.sync.dma_start(bCA[:, 0], b_d["n"][0:128])
        bCB = const.tile([128, 1], F32, tag="bCB")
        nc.sync.dma_start(bCB[0:64, 0], b_d["n"][128:192])

        ident = const.tile([128, 128], F32, tag="ident")
        nc.sync.dma_start(ident[:], id_d)
        identr = const.tile([128, 128], F32R, tag="identr")
        nc.gpsimd.dma_start(identr[:], id_d)

        # --- per-pipeline state tiles ------------------------------------
        memA, memB = {}, {}
        rmemA, rmemB = {}, {}
        zf32 = state.tile([128, WPADP], F32, tag="zf32", name="zf32")
        nc.gpsimd.memset(zf32[:], 0.0)
        for p in range(NP):
            for i in range(2):
                memA[p, i] = state.tile([128, WPADP], F32R, tag=f"memA{p}{i}",
                                        name=f"memA{p}{i}")
                memB[p, i] = state.tile([128, WPADP], F32R, tag=f"memB{p}{i}",
                                        name=f"memB{p}{i}")
            rmemA[p] = state.tile([128, WPADP], F32R, tag=f"rmemA{p}",
                                  name=f"rmemA{p}")
            rmemB[p] = state.tile([128, WPADP], F32R, tag=f"rmemB{p}",
                                  name=f"rmemB{p}")
            for t in (memA[p, 0], memA[p, 1], memB[p, 0], memB[p, 1],
                      rmemA[p], rmemB[p]):
                nc.vector.tensor_copy(t[:], zf32[:])

        # --- input transform: x[b,l,c] -> mem[p][c, 2 + 2l + j] ----------
        for b in range(BLOC):
            p, j = divmod(b, 2)
            xb = tmp.tile([L, C], F32, tag="xload")
            nc.sync.dma_start(xb[:], x_d[b])
            ps = psum.tile([128, L], F32, tag="tpF32")
            nc.tensor.transpose(ps[:], xb[:, 0:128], ident[:])
            nc.vector.tensor_copy(memA[p, 0][:, 2 + j: 2 + j + 2 * L: 2], ps[:])
            ps2 = psum.tile([128, L], F32, tag="tpF32")
            nc.tensor.transpose(ps2[0:64, :], xb[:, 128:192], ident[:])
            nc.vector.tensor_copy(memB[p, 0][0:64, 2 + j: 2 + j + 2 * L: 2],
                                  ps2[0:64, :])
        for p in range(NP):
            nc.vector.tensor_copy(memB[p, 0][64:128, 0:TOKP],
                                  memB[p, 0][0:64, 2:2 + TOKP])

        # --- recurrence --------------------------------------------------
        MWIN = ((0, 0), (1, 2), (2, 4))

        cur = 0
        pt = {}   # psum tiles per pipeline, this step
        sig = {}  # activation tiles per pipeline

        def emit_rg(p):
            mA, mB = memA[p, cur], memB[p, cur]
            pT0 = psum.tile([128, TOKP], F32, tag=f"pT0_{p}", name=f"pT0_{p}")
            pT1 = psum.tile([128, TOKP], F32, tag=f"pT1_{p}", name=f"pT1_{p}")
            pT2 = psum.tile([128, TOKP], F32, tag=f"pT2_{p}", name=f"pT2_{p}")
            pt[p] = (pT0, pT1, pT2)

            def cha(pp, wts):
                for c, off in MWIN:
                    nc.tensor.matmul(pp[:], wts[c][:], mA[:, off:off + TOKP],
                                     start=(c == 0), stop=False)

            def chb(pp, wts):
                nc.tensor.matmul(pp[:], wts[3][:], mB[:, 0:TOKP],
                                 start=False, stop=False)
                nc.tensor.matmul(pp[:], wts[4][:], mB[:, 2:2 + TOKP],
                                 start=False, stop=True)

            cha(pT0, wt["T0"])
            cha(pT2, wt["T2"])
            chb(pT0, wt["T0"])
            chb(pT2, wt["T2"])
            cha(pT1, wt["T1"])
            chb(pT1, wt["T1"])

        def emit_sig(p):
            pT0, pT1, pT2 = pt[p]
            sA = act.tile([128, TOKP], F32R, tag=f"sA{p}", name=f"sA{p}")
            nc.scalar.activation(sA[:], pT0[:], AF.Sigmoid, bias=bA[:, 0:1])
            sBr = act.tile([64, TOKP], F32R, tag=f"sBr{p}", name=f"sBr{p}")
            nc.scalar.activation(sBr[:], pT2[0:64, :], AF.Sigmoid,
                                 bias=bB[0:64, 0:1])
            sGb = act.tile([64, TOKP], F32R, tag=f"sGb{p}", name=f"sGb{p}")
            nc.scalar.activation(sGb[:], pT2[64:128, :], AF.Sigmoid,
                                 bias=bB[64:128, 0:1])
            sG = act.tile([128, TOKP], F32R, tag=f"sG{p}", name=f"sG{p}")
            nc.scalar.activation(sG[:], pT1[:], AF.Sigmoid, bias=bG[:, 0:1])
            sig[p] = (sA, sBr, sGb, sG)

        def emit_rmem_gps(p):
            sA, sBr, _, _ = sig[p]
            mA, mB = memA[p, cur], memB[p, cur]
            nc.gpsimd.tensor_mul(rmemA[p][:, 2:2 + TOKP], sA[:],
                                 mA[:, 2:2 + TOKP])
            nc.gpsimd.tensor_mul(rmemB[p][0:64, 2:2 + TOKP], sBr[:],
                                 mB[0:64, 2:2 + TOKP])
            nc.gpsimd.tensor_copy(rmemB[p][64:128, 0:TOKP],
                                  rmemB[p][0:64, 2:2 + TOKP])

        def emit_rmem_v(p):
            sA, sBr, _, _ = sig[p]
            mA, mB = memA[p, cur], memB[p, cur]
            nc.gpsimd.tensor_mul(rmemA[p][:, 2:2 + TOKP], sA[:],
                                 mA[:, 2:2 + TOKP])
            nc.vector.tensor_mul(rmemB[p][0:64, 2:2 + TOKP], sBr[:],
                                 mB[0:64, 2:2 + TOKP])
            nc.vector.tensor_copy(rmemB[p][64:128, 0:TOKP],
                                  rmemB[p][0:64, 2:2 + TOKP])

        def emit_u(p):
            _, _, sGb, sG = sig[p]
            mA, mB = memA[p, cur], memB[p, cur]
            uA = tmp.tile([128, TOKP], F32R, tag=f"uA{p}", name=f"uA{p}")
            nc.gpsimd.tensor_mul(uA[:], sG[:], mA[:, 0:TOKP])
            uB = tmp.tile([64, TOKP], F32R, tag=f"uB{p}", name=f"uB{p}")
            nc.gpsimd.tensor_mul(uB[:], sGb[:], mB[0:64, 0:TOKP])
            return uA, uB

        def emit_cand(p):
            pC0 = psum.tile([128, TOKP], F32, tag=f"pT0_{p}", name=f"pC0_{p}")
            pC1 = psum.tile([128, TOKP], F32, tag=f"pT1_{p}", name=f"pC1_{p}")
            for c, off in MWIN:
                nc.tensor.matmul(pC0[:], wt["C0"][c][:],
                                 rmemA[p][:, off:off + TOKP],
                                 start=(c == 0), stop=False)
            nc.tensor.matmul(pC0[:], wt["C0"][3][:], rmemB[p][:, 0:TOKP],
                             start=False, stop=False)
            nc.tensor.matmul(pC0[:], wt["C0"][4][:], rmemB[p][:, 2:2 + TOKP],
                             start=False, stop=True)
            for c, off in MWIN:
                nc.tensor.matmul(pC1[:], wt["C1"][c][:],
                                 rmemA[p][:, off:off + TOKP],
                                 start=(c == 0), stop=False)
            nc.tensor.matmul(pC1[:], wt["C1"][3][:], rmemB[p][:, 0:TOKP],
                             start=False, stop=False)
            nc.tensor.matmul(pC1[:], wt["C1"][4][:], rmemB[p][:, 2:2 + TOKP],
                             start=False, stop=True)
            return pC0, pC1

        def emit_tanh(p, pC0, pC1):
            cA = act.tile([128, TOKP], F32R, tag=f"cA{p}", name=f"cA{p}")
            nc.scalar.activation(cA[:], pC0[:], AF.Tanh, bias=bCA[:, 0:1])
            cB = act.tile([64, TOKP], F32R, tag=f"cB{p}", name=f"cB{p}")
            nc.scalar.activation(cB[:], pC1[0:64, :], AF.Tanh,
                                 bias=bCB[0:64, 0:1])
            return cA, cB

        def emit_combine(p, uA, uB, cA, cB):
            _, _, sGb, sG = sig[p]
            nA, nB = memA[p, 1 - cur], memB[p, 1 - cur]
            qA = tmp.tile([128, TOKP], F32R, tag=f"qA{p}", name=f"qA{p}")
            nc.vector.scalar_tensor_tensor(qA[:], sG[:], 1.0, cA[:],
                                           op0=SUB, op1=MULT)
            nc.vector.tensor_sub(nA[:, 2:2 + TOKP], uA[:], qA[:])
            qB = tmp.tile([64, TOKP], F32R, tag=f"qB{p}", name=f"qB{p}")
            nc.vector.scalar_tensor_tensor(qB[:], sGb[:], 1.0, cB[:],
                                           op0=SUB, op1=MULT)
            nc.vector.tensor_sub(nB[0:64, 2:2 + TOKP], uB[:], qB[:])
            nc.vector.tensor_copy(nB[64:128, 0:TOKP], nB[0:64, 2:2 + TOKP])

        for t in range(steps):
            emit_rg(0)
            emit_rg(1)
            emit_sig(0)
            emit_rmem_gps(0)
            emit_sig(1)
            emit_rmem_v(1)
            u0 = emit_u(0)
            c0psum = emit_cand(0)
            ct0 = emit_tanh(0, *c0psum)
            emit_combine(0, *u0, *ct0)
            c1psum = emit_cand(1)
            ct1 = emit_tanh(1, *c1psum)
            u1 = emit_u(1)
            emit_combine(1, *u1, *ct1)
            cur = 1 - cur

        # --- output transform -------------------------------------------
        for b in range(BLOC):
            p, j = divmod(b, 2)
            osb = tmp.tile([L, C], F32, tag="oload")
            ps = psum.tile([L, 128], F32R, tag="tpR")
            nc.tensor.transpose(ps[:], memA[p, cur][:, 2 + j: 2 + j + 2 * L: 2],
                                identr[:])
            nc.vector.tensor_copy(osb[:, 0:128], ps[:])
            ps2 = psum.tile([L, 128], F32R, tag="tpR")
            nc.tensor.transpose(ps2[:, 0:64],
                                memB[p, cur][0:64, 2 + j: 2 + j + 2 * L: 2],
                                identr[0:64, 0:64])
            nc.vector.tensor_copy(osb[:, 128:192], ps2[:, 0:64])
            nc.sync.dma_start(out_d[b], osb[:])

    nc.compile()
    return nc


_built = {}


def _get(steps=STEPS):
    if steps not in _built:
        _built[steps] = build(steps)
    return _built[steps]


def kernel(x, w_reset, b_reset, w_gate, b_gate, w_cand, b_cand, steps=STEPS,
           trace=False):
    nc = _get(steps)
    ident = np.eye(128, dtype=np.float32)
    base = {"w_reset": np.asarray(w_reset, np.float32),
            "b_reset": np.asarray(b_reset, np.float32),
            "w_gate": np.asarray(w_gate, np.float32),
            "b_gate": np.asarray(b_gate, np.float32),
            "w_cand": np.asarray(w_cand, np.float32),
            "b_cand": np.asarray(b_cand, np.float32),
            "ident": ident}
    x = np.asarray(x, np.float32)
    in_maps = [dict(base, x=np.ascontiguousarray(x[i * BLOC:(i + 1) * BLOC]))
               for i in range(NCORES)]
    res = run_bass_kernel_spmd(nc, in_maps, core_ids=list(range(NCORES)),
                               trace=trace)
    out = np.concatenate([res.results[i]["out"] for i in range(NCORES)], axis=0)
    if trace:
        return out, res
    return out
